# revision 6
# baseline (speedup 1.0000x reference)
"""Bass/Trainium2 kernel for nn_CustomPooling (segment_reduce, masked mean pooling).

Reference computation:
  hs = mean(hidden_states[-4:], axis=0)                      # [B,S,H]
  valid = before_pad & ~CLS & ~SEP & attention
  term_mean = sum_s(hs * term_mask) / sum(term_mask)         # [B,H]
  text_mean = sum_s(hs * text_mask) / sum(text_mask)         # [B,H]
  out = concat([term_mean, text_mean], -1)                   # [B,2H]

Strategy:
  - Only the last 4 layers are ever read (201MB of the 654MB input).
  - The [B,S] int masks reduce to binary {0,1} per-(b,s) weights; the
    1/(4*count) scale is applied to the tiny [B,2H] result on the host, so
    the device work is a pure masked sum over (layer, s):
      acc[b, m*H + h] = sum_{l,s} hs[l,b,s,h] * mask[b,s,m]
  - That reduction is a TensorE matmul with the [128,2,2] binary mask slice
    stationary and hs [128,2,N] moving in fp8 DoubleRow perf mode (256-deep
    contraction per matmul, 2 elem/partition/cycle), accumulated in fp32
    PSUM over 2 double-s-chunks x 4 layers. Data ships as fp8_e4m3 ({0,1}
    masks exact); quantization uses error diffusion along each group's
    (l,s) reduction chain so group sums keep ~3e-3 relative accuracy. This
    halves DMA bytes vs fp16 (the kernel is HBM/fabric-DMA-bound at
    ~430GB/s/core) and doubles PE throughput.
  - Data parallel over B: 8 cores x 4 batches, no collectives.
  - Host pre-swizzles each (batch, layer-pair) into one contiguous
    [128, 6152] fp8 blob (its own weight copy appended) so each tile is
    ONE ~790KB DMA and every matmul waits on exactly one DMA semaphore
    (this toolchain accepts a single sync wait per instruction). The 8 hs
    DMAs alternate between the two HWDGE rings (sync/scalar) to keep all
    16 SDMA engines latency-hidden; both output stores ride the sync ring
    (HWDGE, ~0.6us completion receipt vs ~2us SWDGE), emitted after all
    hs triggers so they never stall an hs DMA, with ring FIFO ordering the
    bulk store before the final store that the exit drain waits on.
"""

import os

import numpy as np

# Hardcoded problem shape (kernel.py must be self-contained).
L, B, S, H = 13, 32, 512, 768
N_LAYERS = 4          # layers -4..-1
N_CORES = 8
B_SHARD = B // N_CORES          # 4 batches per core
N_DCH = S // 256                # 2 double-s-chunks of 256 (DoubleRow contraction)
# Mask region: col = i*16 + d*2 + m (32B; dual-fp8 ldweights needs the
# ktile stride of the stationary AP 16B-aligned), zeros elsewhere.
W_COLS = 32
# Bulk batches (0..2) ship as two half-blobs (2 layers each); the tail
# batch ships as four quarter-blobs (1 layer) so the last-arriving tile
# only needs ~0.7us of matmuls after the final DMA lands.
HALF_HS = 2 * N_DCH * 2 * H              # 6144
HALF_COLS = HALF_HS + W_COLS             # 6152
QUART_HS = N_DCH * 2 * H                 # 3072
QUART_COLS = QUART_HS + W_COLS           # 3080
CLS_ID, SEP_ID, PAD_ID = 101, 102, 0
DIFF_CHAINS = 8       # parallel error-diffusion chains per (b,h,group)
N_WARMUP = 9          # PE clock-ramp dummy matmuls

_CACHED = {}


def _build_bass():
    import concourse.bass as bass
    import concourse.tile as tile
    from concourse import mybir

    f8 = mybir.dt.float8e4
    f32 = mybir.dt.float32
    DR = mybir.MatmulPerfMode.DoubleRow
    nc = bass.Bass()

    # Per-core inputs (host-preswizzled fp8 blobs, masks appended to each):
    #   hsa[b, hf, p, (l2 d) * 1536 + i*768 + n], b in 0..2  (two half-blobs)
    #   hsb[l, p, d*1536 + i*768 + n]                        (batch 3, per layer)
    # where row p and ktile i encode s = d*256 + i*128 + p. Mask cols at
    # hs_end + i*16 + d*2 + m (16B ktile stride for dual-fp8 ldweights).
    hsa = nc.dram_tensor("hsa", [3, 2, 128, HALF_COLS], f8, kind="ExternalInput")
    hsb0 = nc.dram_tensor("hsb0", [2, 128, QUART_COLS], f8, kind="ExternalInput")
    hsb = nc.dram_tensor("hsb", [N_LAYERS, 128, QUART_COLS], f8, kind="ExternalInput")
    out = nc.dram_tensor("out", [B_SHARD, 2 * H], f32, kind="ExternalOutput")

    dma_idx = [0]

    def hs_dma(out_ap, in_ap):
        eng = nc.sync if dma_idx[0] % 2 == 0 else nc.scalar
        dma_idx[0] += 1
        eng.dma_start(out=out_ap, in_=in_ap)

    def blob_mms(t, hs_base, w_base, n_layers):
        """(lhsT, rhs_A, rhs_B) per (layer, dchunk) of one blob."""
        res = []
        wv = t[:, w_base : w_base + W_COLS].rearrange("p (i x) -> p i x", i=2)
        for l2 in range(n_layers):
            for d in range(N_DCH):
                lhsT = wv[:, :, d * 2 : d * 2 + 2]
                col0 = hs_base + (l2 * N_DCH + d) * 2 * H
                hsv = t[:, col0 : col0 + 2 * H].rearrange("p (i n) -> p i n", i=2)
                res.append((lhsT, hsv[:, :, 0:512], hsv[:, :, 512:H]))
        return res

    with tile.TileContext(nc) as tc:
        with (
            tc.tile_pool(name="hs_pool", bufs=5) as hs_pool,
            tc.tile_pool(name="hsq_pool", bufs=6) as hsq_pool,
            tc.tile_pool(name="warm_pool", bufs=1) as warm_pool,
            tc.tile_pool(name="out_pool", bufs=1) as out_pool,
            tc.tile_pool(name="psum", bufs=4, space="PSUM") as psum_pool,
        ):
            out_tile = out_pool.tile([2, B_SHARD * H], f32)

            # PE warmup: the PE runs its first ~8 matmuls at half rate
            # (clock ramp). Burn that ramp on dummy matmuls over garbage
            # SBUF before the first hs blob lands; their PSUM bank is
            # start=True-overwritten on reuse, so values never escape.
            warm = warm_pool.tile([128, 1056], f8, tag="warm")
            nc.vector.memset(warm[:], 0.0)
            warm_psum = psum_pool.tile([2, 512], f32, tag="psum_a")
            w_lhsT = warm[:, 1024:1056].rearrange("p (i x) -> p i x", i=2)[:, :, 0:2]
            w_rhs = warm[:, 0:1024].rearrange("p (i n) -> p i n", i=2)
            for _ in range(N_WARMUP):
                nc.tensor.matmul(warm_psum[:, :], w_lhsT, w_rhs,
                                 start=True, stop=True, perf_mode=DR)

            for b in range(B_SHARD):
                mm_args = []
                last_quarter = []
                if b == 0:
                    # First two blobs are single-layer quarters, one per
                    # HWDGE ring, so the PE starts ~2us earlier than a
                    # half-blob would allow.
                    for l in range(2):
                        t = hsq_pool.tile([128, QUART_COLS], f8, tag="hsq")
                        hs_dma(t[:], hsb0[l])
                        mm_args += blob_mms(t, 0, QUART_HS, 1)
                    t = hs_pool.tile([128, HALF_COLS], f8, tag="hs")
                    hs_dma(t[:], hsa[0, 1])
                    mm_args += blob_mms(t, 0, HALF_HS, 2)
                elif b < 3:
                    for hf in range(2):
                        t = hs_pool.tile([128, HALF_COLS], f8, tag="hs")
                        hs_dma(t[:], hsa[b, hf])
                        mm_args += blob_mms(t, 0, HALF_HS, 2)
                else:
                    for l in range(N_LAYERS):
                        t = hsq_pool.tile([128, QUART_COLS], f8, tag="hsq")
                        hs_dma(t[:], hsb[l])
                        mms = blob_mms(t, 0, QUART_HS, 1)
                        if l == N_LAYERS - 1:
                            last_quarter = mms
                        else:
                            mm_args += mms
                    # Both output stores ride the sync HWDGE ring, emitted
                    # after every hs trigger so they never stall one; ring
                    # FIFO orders bulk before the drain-covered final store.
                    # Bulk (b0..b2) hides under b3's matmuls.
                    nc.sync.dma_start(
                        out=out[0:3].rearrange("b (m h) -> m b h", m=2),
                        in_=out_tile[:, 0 : 3 * H].rearrange(
                            "m (b h) -> m b h", b=3
                        ),
                    )

                # Interleaved bank-A (N=512) / bank-B (N=256) groups in
                # separate PSUM banks. In the final quarter blob the B
                # group closes first so its copy overlaps A's last matmuls.
                psum_a = psum_pool.tile([2, 512], f32, tag="psum_a")
                psum_b = psum_pool.tile([2, H - 512], f32, tag="psum_b")
                na = nb = len(mm_args) + len(last_quarter)
                ia = ib = 0
                for lhsT, rhs_a, rhs_b in mm_args:
                    nc.tensor.matmul(psum_a[:, :], lhsT, rhs_a,
                                     start=ia == 0, stop=ia == na - 1,
                                     perf_mode=DR)
                    ia += 1
                    nc.tensor.matmul(psum_b[:, :], lhsT, rhs_b,
                                     start=ib == 0, stop=ib == nb - 1,
                                     perf_mode=DR)
                    ib += 1
                for lhsT, rhs_a, rhs_b in last_quarter:
                    nc.tensor.matmul(psum_b[:, :], lhsT, rhs_b,
                                     start=ib == 0, stop=ib == nb - 1,
                                     perf_mode=DR)
                    ib += 1
                for lhsT, rhs_a, rhs_b in last_quarter:
                    nc.tensor.matmul(psum_a[:, :], lhsT, rhs_a,
                                     start=ia == 0, stop=ia == na - 1,
                                     perf_mode=DR)
                    ia += 1
                nc.vector.tensor_copy(
                    out=out_tile[:, b * H + 512 : (b + 1) * H], in_=psum_b[:, :]
                )
                nc.vector.tensor_copy(
                    out=out_tile[:, b * H : b * H + 512], in_=psum_a[:, :]
                )

            # Final (b3) store: last DMACopy in program order; the exit
            # drain waits on its completion sem (see _fix_drain_waits).
            nc.sync.dma_start(
                out=out[3:4].rearrange("b (m h) -> m b h", m=2),
                in_=out_tile[:, 3 * H : 4 * H].rearrange(
                    "m (b h) -> m b h", b=1
                ),
            )

    _fix_drain_waits(nc)
    return nc


def _fix_drain_waits(nc):
    """This container's walrus accepts only ONE sync wait per instruction.

    Two Tile-emitted multi-waits are redundant here and get stripped:

    1. Exit drains aggregate one wait per live semaphore; every semaphore
       except the final out-DMA's is transitively ordered before the drain
       (matmuls wait on hs DMAs -> PE; copies wait on PE -> DVE; the out
       DMAs wait on DVE and complete in sync-ring FIFO order ending with
       the final store), so drains keep only the final-store wait.
    2. The 12 HWDGE DMAs wrap the 8 DMAHW sem lanes, so the two out stores
       carry a lane-reuse guard wait next to their DVE wait. The guard is
       implied: the lane's previous hs DMA was already waited on by the
       matmuls that precede the DVE copies the store waits on.
    """
    import bass_rust

    f = nc.m.functions[0]
    # update-sem of the last DMACopy in program order (the final out store)
    last_dma_sem = None
    for bb in f.blocks:
        for ins in bb.instructions:
            if type(ins).__name__ == "InstDMACopy":
                ups = ins.sync_info.on_update
                if ups:
                    last_dma_sem = ups[-1].ant_name

    for bb in f.blocks:
        for ins in bb.instructions:
            nm = type(ins).__name__
            si = ins.sync_info
            if si is None:
                continue
            waits = list(si.on_wait)
            if len(waits) <= 1:
                continue
            if nm == "InstDrain":
                keep = [w for w in waits if w.ant_name == last_dma_sem]
            elif nm == "InstDMACopy":
                keep = [w for w in waits if not w.ant_name.startswith("DMAHW")]
            else:
                continue
            assert len(keep) == 1, (nm, [w.ant_name for w in waits])
            ins.sync_info = bass_rust.SyncInfo(
                on_wait=keep, on_update=list(si.on_update)
            )


def _host_masks(input_ids, attention_mask, token_type_ids):
    ids = np.asarray(input_ids)
    am = np.asarray(attention_mask)
    tt = np.asarray(token_type_ids)

    not_pad = ids != PAD_ID
    before_pad = np.cumprod(not_pad.astype(np.int64), axis=1).astype(bool)
    valid = before_pad & (ids != CLS_ID) & (ids != SEP_ID) & (am == 1)
    term = valid & (tt == 0)
    text = valid & (tt == 1)
    masks = np.stack([term, text], axis=-1)  # [B, S, 2] bool
    counts = masks.sum(axis=1).astype(np.float64)  # [B, 2]
    return masks, counts


def _diffused_fp8(hs4, masks):
    """Quantize to fp8_e4m3 with error diffusion along each group's (l,s)
    reduction chain: the rounding residual of each masked element is carried
    into the next masked element of the same (b, h, group) chain, so each
    group's quantization errors telescope to ~1 ulp instead of a sqrt(N)
    random walk. DIFF_CHAINS stripes s into parallel chains (vectorizing the
    host loop) at a sqrt(DIFF_CHAINS) error cost; measured group-sum rel err
    ~2.4e-3 vs the 2e-2 gate. Device-side sum order doesn't matter -- only
    the group SUM of the quantized values.
    """
    import ml_dtypes

    F8 = ml_dtypes.float8_e4m3
    K = DIFF_CHAINS
    q = np.empty(hs4.shape, dtype=F8)  # [4, B, S, H]
    gt_all = masks[:, :, 0]  # [B, S]
    gx_all = masks[:, :, 1]
    carry_t = np.zeros((K, B, H), dtype=np.float32)
    carry_x = np.zeros((K, B, H), dtype=np.float32)
    for l in range(N_LAYERS):
        for j in range(S // K):
            sblk = slice(j * K, (j + 1) * K)
            gt = gt_all[:, sblk].T[:, :, None]  # [K, B, 1]
            gx = gx_all[:, sblk].T[:, :, None]
            t = hs4[l, :, sblk, :].transpose(1, 0, 2) + np.where(
                gt, carry_t, carry_x
            )  # [K, B, H]
            qv = t.astype(F8)
            q[l, :, sblk, :] = qv.transpose(1, 0, 2)
            resid = t - qv.astype(np.float32)
            carry_t = np.where(gt, resid, carry_t)
            carry_x = np.where(gx, resid, carry_x)
    return q


def kernel(hidden_states, input_ids, attention_mask, token_type_ids):
    from concourse.bass_utils import run_bass_kernel_spmd

    hs_full = np.asarray(hidden_states)
    masks, counts = _host_masks(input_ids, attention_mask, token_type_ids)

    q = _diffused_fp8(hs_full[L - N_LAYERS :].astype(np.float32), masks)
    F8 = q.dtype

    # Half-blobs [B, hf, p, ((l2 d) i n)] and quarter-blobs [B, l, p, (d i n)]
    # with s = d*256 + i*128 + p.
    half = np.empty((B, 2, 128, HALF_COLS), dtype=F8)
    half[:, :, :, :HALF_HS] = (
        q.reshape(2, 2, B, N_DCH, 2, 128, H)
        .transpose(2, 0, 5, 1, 3, 4, 6)
        .reshape(B, 2, 128, HALF_HS)
    )
    quart = np.empty((B, N_LAYERS, 128, QUART_COLS), dtype=F8)
    quart[:, :, :, :QUART_HS] = (
        q.reshape(N_LAYERS, B, N_DCH, 2, 128, H)
        .transpose(1, 0, 4, 2, 3, 5)
        .reshape(B, N_LAYERS, 128, QUART_HS)
    )
    wv = np.zeros((B, 128, 2, 16), dtype=F8)
    wv[:, :, :, 0:4] = (
        masks.reshape(B, N_DCH, 2, 128, 2)
        .transpose(0, 3, 2, 1, 4)          # (b, p, i, d, m)
        .reshape(B, 128, 2, 4)
        .astype(F8)
    )
    wv = wv.reshape(B, 128, W_COLS)
    half[:, :, :, HALF_HS:] = wv[:, None, :, :]
    quart[:, :, :, QUART_HS:] = wv[:, None, :, :]

    in_maps = [
        {
            "hsa": half[i * B_SHARD : i * B_SHARD + 3],
            "hsb0": quart[i * B_SHARD, 0:2],
            "hsb": quart[i * B_SHARD + 3],
        }
        for i in range(N_CORES)
    ]

    if "nc" not in _CACHED:
        _CACHED["nc"] = _build_bass()
    nc = _CACHED["nc"]

    trace = os.environ.get("KERNEL_TRACE", "0") == "1"
    if trace:
        _install_ntff_hook_shim()
    tmpdir = os.environ.get("KERNEL_TMPDIR") or None
    res = run_bass_kernel_spmd(
        nc, in_maps, core_ids=list(range(N_CORES)), trace=trace, tmpdir=tmpdir
    )
    kernel.last_results = res

    acc = np.concatenate([r["out"] for r in res.results], axis=0)  # [B, 2H]
    # Apply the masked-mean normalization (exact f64 scale, mirrors the
    # reference's sum/count including inf/nan semantics for count==0).
    with np.errstate(divide="ignore", invalid="ignore"):
        scale = 1.0 / (N_LAYERS * counts)  # [B, 2]
    out = acc.reshape(B, 2, H) * scale[:, :, None]
    return out.reshape(B, 2 * H).astype(np.float32)


def _install_ntff_hook_shim():
    """The container's antenv stub lacks axon_hooks, which silently disables
    NTFF profiling under trace=True. Recreate it: a tiny get/set registry plus
    the ctypes hook into libaxon_pjrt.so (same as trn_boot's installer)."""
    import contextlib
    import ctypes
    import sys
    import types

    if "antenv.axon_hooks" in sys.modules:
        return
    so_path = "/opt/axon/libaxon_pjrt.so"
    try:
        lib = ctypes.CDLL(so_path)
    except OSError:
        return
    if not hasattr(lib, "axon_start_nrt_profile"):
        return
    lib.axon_start_nrt_profile.argtypes = [
        ctypes.POINTER(ctypes.c_int64),
        ctypes.c_size_t,
    ]
    lib.axon_start_nrt_profile.restype = ctypes.c_int64
    lib.axon_stop_nrt_profile.argtypes = [ctypes.c_char_p]
    lib.axon_stop_nrt_profile.restype = ctypes.c_int64

    @contextlib.contextmanager
    def _hook(output_dir, device_ids):
        import jax

        jax.devices()
        if device_ids:
            ids = (ctypes.c_int64 * len(device_ids))(*device_ids)
            rc = lib.axon_start_nrt_profile(ids, len(device_ids))
        else:
            rc = lib.axon_start_nrt_profile(None, 0)
        if rc != 0:
            raise RuntimeError(f"axon_start_nrt_profile rc={rc}")
        try:
            yield
        finally:
            n = lib.axon_stop_nrt_profile(str(output_dir).encode())
            print(f"profile: {n} file(s) written to {output_dir}", file=sys.stderr)

    mod = types.ModuleType("antenv.axon_hooks")
    _state = {"hook": _hook}
    mod.set_axon_ntff_profile_hook = lambda h: _state.__setitem__("hook", h)
    mod.get_axon_ntff_profile_hook = lambda: _state["hook"]
    sys.modules["antenv.axon_hooks"] = mod
    import antenv

    antenv.axon_hooks = mod


# revision 7
# speedup vs baseline: 1.0378x; 1.0378x over previous
"""Bass/Trainium2 kernel for nn_CustomPooling (segment_reduce, masked mean pooling).

Reference computation:
  hs = mean(hidden_states[-4:], axis=0)                      # [B,S,H]
  valid = before_pad & ~CLS & ~SEP & attention
  term_mean = sum_s(hs * term_mask) / sum(term_mask)         # [B,H]
  text_mean = sum_s(hs * text_mask) / sum(text_mask)         # [B,H]
  out = concat([term_mean, text_mean], -1)                   # [B,2H]

Strategy:
  - Only the last 4 layers are ever read (201MB of the 654MB input).
  - The [B,S] int masks reduce to binary {0,1} per-(b,s) weights; the
    1/(4*count) scale is applied to the tiny [B,2H] result on the host, so
    the device work is a pure masked sum over (layer, s):
      acc[b, m*H + h] = sum_{l,s} hs[l,b,s,h] * mask[b,s,m]
  - That reduction is a TensorE matmul with the [128,2,2] binary mask slice
    stationary and hs [128,2,N] moving in fp8 DoubleRow perf mode (256-deep
    contraction per matmul, 2 elem/partition/cycle), accumulated in fp32
    PSUM over 2 double-s-chunks x 4 layers. Data ships as fp8_e4m3 ({0,1}
    masks exact); quantization uses error diffusion along each group's
    (l,s) reduction chain so group sums keep ~3e-3 relative accuracy. This
    halves DMA bytes vs fp16 (the kernel is HBM/fabric-DMA-bound at
    ~430GB/s/core) and doubles PE throughput.
  - Data parallel over B: 8 cores x 4 batches, no collectives.
  - Host pre-swizzles each (batch, layer-pair) into one contiguous
    [128, 6152] fp8 blob (its own weight copy appended) so each tile is
    ONE DMA and every matmul waits on exactly one DMA semaphore (this
    toolchain accepts a single sync wait per instruction). All hs DMAs
    ride the sync HWDGE ring in consumption order (one ring already runs
    all 16 SDMA engines at the ~430GB/s fabric rate, and ring FIFO makes
    arrival order deterministic); both output stores ride the scalar ring
    (HWDGE, ~0.6us completion receipt vs ~2us SWDGE), with ring FIFO
    ordering the bulk store before the final store the exit drain waits
    on. Dummy warmup matmuls absorb the PE's ~4us half-rate clock ramp
    before the first blob lands.
"""

import os

import numpy as np

# Hardcoded problem shape (kernel.py must be self-contained).
L, B, S, H = 13, 32, 512, 768
N_LAYERS = 4          # layers -4..-1
N_CORES = 8
B_SHARD = B // N_CORES          # 4 batches per core
N_DCH = S // 256                # 2 double-s-chunks of 256 (DoubleRow contraction)
# Mask region: col = i*16 + d*2 + m (32B; dual-fp8 ldweights needs the
# ktile stride of the stationary AP 16B-aligned), zeros elsewhere.
W_COLS = 32
# Bulk batches (0..2) ship as two half-blobs (2 layers each); the tail
# batch ships as four quarter-blobs (1 layer) so the last-arriving tile
# only needs ~0.7us of matmuls after the final DMA lands.
HALF_HS = 2 * N_DCH * 2 * H              # 6144
HALF_COLS = HALF_HS + W_COLS             # 6152
QUART_HS = N_DCH * 2 * H                 # 3072
QUART_COLS = QUART_HS + W_COLS           # 3080
CLS_ID, SEP_ID, PAD_ID = 101, 102, 0
DIFF_CHAINS = 8       # parallel error-diffusion chains per (b,h,group)
N_WARMUP = 8          # PE clock-ramp dummy matmuls

_CACHED = {}


def _build_bass():
    import concourse.bass as bass
    import concourse.tile as tile
    from concourse import mybir

    f8 = mybir.dt.float8e4
    f32 = mybir.dt.float32
    DR = mybir.MatmulPerfMode.DoubleRow
    nc = bass.Bass()

    # Per-core inputs (host-preswizzled fp8 blobs, masks appended to each):
    #   hsa[b, hf, p, (l2 d) * 1536 + i*768 + n], b in 0..2  (two half-blobs)
    #   hsb[l, p, d*1536 + i*768 + n]                        (batch 3, per layer)
    # where row p and ktile i encode s = d*256 + i*128 + p. Mask cols at
    # hs_end + i*16 + d*2 + m (16B ktile stride for dual-fp8 ldweights).
    hsa = nc.dram_tensor("hsa", [3, 2, 128, HALF_COLS], f8, kind="ExternalInput")
    hsb0 = nc.dram_tensor("hsb0", [2, 128, QUART_COLS], f8, kind="ExternalInput")
    hsb = nc.dram_tensor("hsb", [N_LAYERS, 128, QUART_COLS], f8, kind="ExternalInput")
    out = nc.dram_tensor("out", [B_SHARD, 2 * H], f32, kind="ExternalOutput")

    def hs_dma(out_ap, in_ap):
        # All hs DMAs ride ONE HWDGE ring (sync) in consumption order: ring
        # FIFO makes blob completion order deterministic, and a single ring
        # still spreads each InstDMACopy across all 16 SDMA engines at full
        # ~430GB/s fabric rate. (Two rings with asymmetric loads let the
        # packet scheduler starve one ring and stall the PE.) Stores ride
        # the scalar ring so their DVE waits never stall an hs trigger.
        nc.sync.dma_start(out=out_ap, in_=in_ap)

    def blob_mms(t, hs_base, w_base, n_layers):
        """(lhsT, rhs_A, rhs_B) per (layer, dchunk) of one blob."""
        res = []
        wv = t[:, w_base : w_base + W_COLS].rearrange("p (i x) -> p i x", i=2)
        for l2 in range(n_layers):
            for d in range(N_DCH):
                lhsT = wv[:, :, d * 2 : d * 2 + 2]
                col0 = hs_base + (l2 * N_DCH + d) * 2 * H
                hsv = t[:, col0 : col0 + 2 * H].rearrange("p (i n) -> p i n", i=2)
                res.append((lhsT, hsv[:, :, 0:512], hsv[:, :, 512:H]))
        return res

    with tile.TileContext(nc) as tc:
        with (
            tc.tile_pool(name="hs_pool", bufs=5) as hs_pool,
            tc.tile_pool(name="hsq_pool", bufs=6) as hsq_pool,
            tc.tile_pool(name="warm_pool", bufs=1) as warm_pool,
            tc.tile_pool(name="out_pool", bufs=1) as out_pool,
            tc.tile_pool(name="psum", bufs=4, space="PSUM") as psum_pool,
        ):
            out_tile = out_pool.tile([2, B_SHARD * H], f32)

            # PE warmup: the PE runs its first ~8 matmuls at half rate
            # (clock ramp). Burn that ramp on dummy matmuls over garbage
            # SBUF before the first hs blob lands; their PSUM bank is
            # start=True-overwritten on reuse, so values never escape.
            warm = warm_pool.tile([128, 1056], f8, tag="warm")
            nc.vector.memset(warm[:], 0.0)
            warm_psum = psum_pool.tile([2, 512], f32, tag="psum_a")
            w_lhsT = warm[:, 1024:1056].rearrange("p (i x) -> p i x", i=2)[:, :, 0:2]
            w_rhs = warm[:, 0:1024].rearrange("p (i n) -> p i n", i=2)
            for _ in range(N_WARMUP):
                nc.tensor.matmul(warm_psum[:, :], w_lhsT, w_rhs,
                                 start=True, stop=True, perf_mode=DR)

            for b in range(B_SHARD):
                mm_args = []
                last_quarter = []
                if b == 0:
                    # First two blobs are single-layer quarters, one per
                    # HWDGE ring, so the PE starts ~2us earlier than a
                    # half-blob would allow.
                    for l in range(2):
                        t = hsq_pool.tile([128, QUART_COLS], f8, tag="hsq")
                        hs_dma(t[:], hsb0[l])
                        mm_args += blob_mms(t, 0, QUART_HS, 1)
                    t = hs_pool.tile([128, HALF_COLS], f8, tag="hs")
                    hs_dma(t[:], hsa[0, 1])
                    mm_args += blob_mms(t, 0, HALF_HS, 2)
                elif b < 3:
                    for hf in range(2):
                        t = hs_pool.tile([128, HALF_COLS], f8, tag="hs")
                        hs_dma(t[:], hsa[b, hf])
                        mm_args += blob_mms(t, 0, HALF_HS, 2)
                else:
                    for l in range(N_LAYERS):
                        t = hsq_pool.tile([128, QUART_COLS], f8, tag="hsq")
                        hs_dma(t[:], hsb[l])
                        mms = blob_mms(t, 0, QUART_HS, 1)
                        if l == N_LAYERS - 1:
                            last_quarter = mms
                        else:
                            mm_args += mms
                    # Both output stores ride the sync HWDGE ring, emitted
                    # after every hs trigger so they never stall one; ring
                    # FIFO orders bulk before the drain-covered final store.
                    # Bulk (b0..b2) hides under b3's matmuls.
                    nc.scalar.dma_start(
                        out=out[0:3].rearrange("b (m h) -> m b h", m=2),
                        in_=out_tile[:, 0 : 3 * H].rearrange(
                            "m (b h) -> m b h", b=3
                        ),
                    )

                # Interleaved bank-A (N=512) / bank-B (N=256) groups in
                # separate PSUM banks. In the final quarter blob the B
                # group closes first so its copy overlaps A's last matmuls.
                psum_a = psum_pool.tile([2, 512], f32, tag="psum_a")
                psum_b = psum_pool.tile([2, H - 512], f32, tag="psum_b")
                na = nb = len(mm_args) + len(last_quarter)
                ia = ib = 0
                for lhsT, rhs_a, rhs_b in mm_args:
                    nc.tensor.matmul(psum_a[:, :], lhsT, rhs_a,
                                     start=ia == 0, stop=ia == na - 1,
                                     perf_mode=DR)
                    ia += 1
                    nc.tensor.matmul(psum_b[:, :], lhsT, rhs_b,
                                     start=ib == 0, stop=ib == nb - 1,
                                     perf_mode=DR)
                    ib += 1
                for lhsT, rhs_a, rhs_b in last_quarter:
                    nc.tensor.matmul(psum_b[:, :], lhsT, rhs_b,
                                     start=ib == 0, stop=ib == nb - 1,
                                     perf_mode=DR)
                    ib += 1
                for lhsT, rhs_a, rhs_b in last_quarter:
                    nc.tensor.matmul(psum_a[:, :], lhsT, rhs_a,
                                     start=ia == 0, stop=ia == na - 1,
                                     perf_mode=DR)
                    ia += 1
                nc.vector.tensor_copy(
                    out=out_tile[:, b * H + 512 : (b + 1) * H], in_=psum_b[:, :]
                )
                nc.vector.tensor_copy(
                    out=out_tile[:, b * H : b * H + 512], in_=psum_a[:, :]
                )

            # Final (b3) store: last DMACopy in program order; the exit
            # drain waits on its completion sem (see _fix_drain_waits).
            nc.scalar.dma_start(
                out=out[3:4].rearrange("b (m h) -> m b h", m=2),
                in_=out_tile[:, 3 * H : 4 * H].rearrange(
                    "m (b h) -> m b h", b=1
                ),
            )

    _fix_drain_waits(nc)
    return nc


def _fix_drain_waits(nc):
    """This container's walrus accepts only ONE sync wait per instruction.

    Two Tile-emitted multi-waits are redundant here and get stripped:

    1. Exit drains aggregate one wait per live semaphore; every semaphore
       except the final out-DMA's is transitively ordered before the drain
       (matmuls wait on hs DMAs -> PE; copies wait on PE -> DVE; the out
       DMAs wait on DVE and complete in sync-ring FIFO order ending with
       the final store), so drains keep only the final-store wait.
    2. The 12 HWDGE DMAs wrap the 8 DMAHW sem lanes, so the two out stores
       carry a lane-reuse guard wait next to their DVE wait. The guard is
       implied: the lane's previous hs DMA was already waited on by the
       matmuls that precede the DVE copies the store waits on.
    """
    import bass_rust

    f = nc.m.functions[0]
    # update-sem of the last DMACopy in program order (the final out store)
    last_dma_sem = None
    for bb in f.blocks:
        for ins in bb.instructions:
            if type(ins).__name__ == "InstDMACopy":
                ups = ins.sync_info.on_update
                if ups:
                    last_dma_sem = ups[-1].ant_name

    for bb in f.blocks:
        for ins in bb.instructions:
            nm = type(ins).__name__
            si = ins.sync_info
            if si is None:
                continue
            waits = list(si.on_wait)
            if len(waits) <= 1:
                continue
            if nm == "InstDrain":
                keep = [w for w in waits if w.ant_name == last_dma_sem]
            elif nm == "InstDMACopy":
                keep = [w for w in waits if not w.ant_name.startswith("DMAHW")]
            else:
                continue
            assert len(keep) == 1, (nm, [w.ant_name for w in waits])
            ins.sync_info = bass_rust.SyncInfo(
                on_wait=keep, on_update=list(si.on_update)
            )


def _host_masks(input_ids, attention_mask, token_type_ids):
    ids = np.asarray(input_ids)
    am = np.asarray(attention_mask)
    tt = np.asarray(token_type_ids)

    not_pad = ids != PAD_ID
    before_pad = np.cumprod(not_pad.astype(np.int64), axis=1).astype(bool)
    valid = before_pad & (ids != CLS_ID) & (ids != SEP_ID) & (am == 1)
    term = valid & (tt == 0)
    text = valid & (tt == 1)
    masks = np.stack([term, text], axis=-1)  # [B, S, 2] bool
    counts = masks.sum(axis=1).astype(np.float64)  # [B, 2]
    return masks, counts


def _diffused_fp8(hs4, masks):
    """Quantize to fp8_e4m3 with error diffusion along each group's (l,s)
    reduction chain: the rounding residual of each masked element is carried
    into the next masked element of the same (b, h, group) chain, so each
    group's quantization errors telescope to ~1 ulp instead of a sqrt(N)
    random walk. DIFF_CHAINS stripes s into parallel chains (vectorizing the
    host loop) at a sqrt(DIFF_CHAINS) error cost; measured group-sum rel err
    ~2.4e-3 vs the 2e-2 gate. Device-side sum order doesn't matter -- only
    the group SUM of the quantized values.
    """
    import ml_dtypes

    F8 = ml_dtypes.float8_e4m3
    K = DIFF_CHAINS
    q = np.empty(hs4.shape, dtype=F8)  # [4, B, S, H]
    gt_all = masks[:, :, 0]  # [B, S]
    gx_all = masks[:, :, 1]
    carry_t = np.zeros((K, B, H), dtype=np.float32)
    carry_x = np.zeros((K, B, H), dtype=np.float32)
    for l in range(N_LAYERS):
        for j in range(S // K):
            sblk = slice(j * K, (j + 1) * K)
            gt = gt_all[:, sblk].T[:, :, None]  # [K, B, 1]
            gx = gx_all[:, sblk].T[:, :, None]
            t = hs4[l, :, sblk, :].transpose(1, 0, 2) + np.where(
                gt, carry_t, carry_x
            )  # [K, B, H]
            qv = t.astype(F8)
            q[l, :, sblk, :] = qv.transpose(1, 0, 2)
            resid = t - qv.astype(np.float32)
            carry_t = np.where(gt, resid, carry_t)
            carry_x = np.where(gx, resid, carry_x)
    return q


def kernel(hidden_states, input_ids, attention_mask, token_type_ids):
    from concourse.bass_utils import run_bass_kernel_spmd

    hs_full = np.asarray(hidden_states)
    masks, counts = _host_masks(input_ids, attention_mask, token_type_ids)

    q = _diffused_fp8(hs_full[L - N_LAYERS :].astype(np.float32), masks)
    F8 = q.dtype

    # Half-blobs [B, hf, p, ((l2 d) i n)] and quarter-blobs [B, l, p, (d i n)]
    # with s = d*256 + i*128 + p.
    half = np.empty((B, 2, 128, HALF_COLS), dtype=F8)
    half[:, :, :, :HALF_HS] = (
        q.reshape(2, 2, B, N_DCH, 2, 128, H)
        .transpose(2, 0, 5, 1, 3, 4, 6)
        .reshape(B, 2, 128, HALF_HS)
    )
    quart = np.empty((B, N_LAYERS, 128, QUART_COLS), dtype=F8)
    quart[:, :, :, :QUART_HS] = (
        q.reshape(N_LAYERS, B, N_DCH, 2, 128, H)
        .transpose(1, 0, 4, 2, 3, 5)
        .reshape(B, N_LAYERS, 128, QUART_HS)
    )
    wv = np.zeros((B, 128, 2, 16), dtype=F8)
    wv[:, :, :, 0:4] = (
        masks.reshape(B, N_DCH, 2, 128, 2)
        .transpose(0, 3, 2, 1, 4)          # (b, p, i, d, m)
        .reshape(B, 128, 2, 4)
        .astype(F8)
    )
    wv = wv.reshape(B, 128, W_COLS)
    half[:, :, :, HALF_HS:] = wv[:, None, :, :]
    quart[:, :, :, QUART_HS:] = wv[:, None, :, :]

    in_maps = [
        {
            "hsa": half[i * B_SHARD : i * B_SHARD + 3],
            "hsb0": quart[i * B_SHARD, 0:2],
            "hsb": quart[i * B_SHARD + 3],
        }
        for i in range(N_CORES)
    ]

    if "nc" not in _CACHED:
        _CACHED["nc"] = _build_bass()
    nc = _CACHED["nc"]

    trace = os.environ.get("KERNEL_TRACE", "0") == "1"
    if trace:
        _install_ntff_hook_shim()
    tmpdir = os.environ.get("KERNEL_TMPDIR") or None
    res = run_bass_kernel_spmd(
        nc, in_maps, core_ids=list(range(N_CORES)), trace=trace, tmpdir=tmpdir
    )
    kernel.last_results = res

    acc = np.concatenate([r["out"] for r in res.results], axis=0)  # [B, 2H]
    # Apply the masked-mean normalization (exact f64 scale, mirrors the
    # reference's sum/count including inf/nan semantics for count==0).
    with np.errstate(divide="ignore", invalid="ignore"):
        scale = 1.0 / (N_LAYERS * counts)  # [B, 2]
    out = acc.reshape(B, 2, H) * scale[:, :, None]
    return out.reshape(B, 2 * H).astype(np.float32)


def _install_ntff_hook_shim():
    """The container's antenv stub lacks axon_hooks, which silently disables
    NTFF profiling under trace=True. Recreate it: a tiny get/set registry plus
    the ctypes hook into libaxon_pjrt.so (same as trn_boot's installer)."""
    import contextlib
    import ctypes
    import sys
    import types

    if "antenv.axon_hooks" in sys.modules:
        return
    so_path = "/opt/axon/libaxon_pjrt.so"
    try:
        lib = ctypes.CDLL(so_path)
    except OSError:
        return
    if not hasattr(lib, "axon_start_nrt_profile"):
        return
    lib.axon_start_nrt_profile.argtypes = [
        ctypes.POINTER(ctypes.c_int64),
        ctypes.c_size_t,
    ]
    lib.axon_start_nrt_profile.restype = ctypes.c_int64
    lib.axon_stop_nrt_profile.argtypes = [ctypes.c_char_p]
    lib.axon_stop_nrt_profile.restype = ctypes.c_int64

    @contextlib.contextmanager
    def _hook(output_dir, device_ids):
        import jax

        jax.devices()
        if device_ids:
            ids = (ctypes.c_int64 * len(device_ids))(*device_ids)
            rc = lib.axon_start_nrt_profile(ids, len(device_ids))
        else:
            rc = lib.axon_start_nrt_profile(None, 0)
        if rc != 0:
            raise RuntimeError(f"axon_start_nrt_profile rc={rc}")
        try:
            yield
        finally:
            n = lib.axon_stop_nrt_profile(str(output_dir).encode())
            print(f"profile: {n} file(s) written to {output_dir}", file=sys.stderr)

    mod = types.ModuleType("antenv.axon_hooks")
    _state = {"hook": _hook}
    mod.set_axon_ntff_profile_hook = lambda h: _state.__setitem__("hook", h)
    mod.get_axon_ntff_profile_hook = lambda: _state["hook"]
    sys.modules["antenv.axon_hooks"] = mod
    import antenv

    antenv.axon_hooks = mod


# revision 9
# speedup vs baseline: 1.0959x; 1.0559x over previous
"""Bass/Trainium2 kernel for nn_CustomPooling (segment_reduce, masked mean pooling).

Reference computation:
  hs = mean(hidden_states[-4:], axis=0)                      # [B,S,H]
  valid = before_pad & ~CLS & ~SEP & attention
  term_mean = sum_s(hs * term_mask) / sum(term_mask)         # [B,H]
  text_mean = sum_s(hs * text_mask) / sum(text_mask)         # [B,H]
  out = concat([term_mean, text_mean], -1)                   # [B,2H]

Strategy:
  - Only the last 4 layers are ever read (201MB of the 654MB input).
  - The [B,S] int masks reduce to binary {0,1} per-(b,s) weights; the
    1/(4*count) scale is applied to the tiny [B,2H] result on the host, so
    the device work is a pure masked sum over (layer, s):
      acc[b, m*H + h] = sum_{l,s} hs[l,b,s,h] * mask[b,s,m]
  - That reduction is a TensorE matmul with the [128,2,2] binary mask slice
    stationary and hs [128,2,N] moving in fp8 DoubleRow perf mode (256-deep
    contraction per matmul, 2 elem/partition/cycle), accumulated in fp32
    PSUM over 2 double-s-chunks x 4 layers. Data ships as fp8_e4m3 ({0,1}
    masks exact); quantization uses error diffusion along each group's
    (l,s) reduction chain so group sums keep ~3e-3 relative accuracy. This
    halves DMA bytes vs fp16 (the kernel is HBM/fabric-DMA-bound at
    ~430GB/s/core) and doubles PE throughput.
  - Data parallel over B: 8 cores x 4 batches, no collectives.
  - Host pre-swizzles each (batch, layer-pair) into one contiguous
    [128, 6152] fp8 blob (its own weight copy appended) so each tile is
    ONE DMA and every matmul waits on exactly one DMA semaphore (this
    toolchain accepts a single sync wait per instruction). All hs DMAs
    ride the sync HWDGE ring in consumption order (one ring already runs
    all 16 SDMA engines at the ~430GB/s fabric rate, and ring FIFO makes
    arrival order deterministic); both output stores ride the scalar ring
    (HWDGE, ~0.6us completion receipt vs ~2us SWDGE), with ring FIFO
    ordering the bulk store before the final store the exit drain waits
    on. Dummy warmup matmuls absorb the PE's ~4us half-rate clock ramp
    before the first blob lands.
"""

import os

import numpy as np

# Hardcoded problem shape (kernel.py must be self-contained).
L, B, S, H = 13, 32, 512, 768
N_LAYERS = 4          # layers -4..-1
N_CORES = 8
B_SHARD = B // N_CORES          # 4 batches per core
N_DCH = S // 256                # 2 double-s-chunks of 256 (DoubleRow contraction)
# Mask region: col = i*16 + d*2 + m (32B; dual-fp8 ldweights needs the
# ktile stride of the stationary AP 16B-aligned), zeros elsewhere.
W_COLS = 32
# Bulk batches (0..2) ship as two half-blobs (2 layers each); the tail
# batch ships as four quarter-blobs (1 layer) so the last-arriving tile
# only needs ~0.7us of matmuls after the final DMA lands.
HALF_HS = 2 * N_DCH * 2 * H              # 6144
HALF_COLS = HALF_HS + W_COLS             # 6152
QUART_HS = N_DCH * 2 * H                 # 3072
QUART_COLS = QUART_HS + W_COLS           # 3104
FULL_HS = 4 * N_DCH * 2 * H              # 12288
FULL_COLS = FULL_HS + W_COLS             # 12320 (12.3KB rows: max DMA rate)
CLS_ID, SEP_ID, PAD_ID = 101, 102, 0
DIFF_CHAINS = 8       # parallel error-diffusion chains per (b,h,group)
N_WARMUP = 8          # PE clock-ramp dummy matmuls

_CACHED = {}


def _build_bass():
    import concourse.bass as bass
    import concourse.tile as tile
    from concourse import mybir

    f8 = mybir.dt.float8e4
    f32 = mybir.dt.float32
    DR = mybir.MatmulPerfMode.DoubleRow
    nc = bass.Bass()

    # Per-core inputs (host-preswizzled fp8 blobs, masks appended to each):
    #   hsa[b, hf, p, (l2 d) * 1536 + i*768 + n], b in 0..2  (two half-blobs)
    #   hsb[l, p, d*1536 + i*768 + n]                        (batch 3, per layer)
    # where row p and ktile i encode s = d*256 + i*128 + p. Mask cols at
    # hs_end + i*16 + d*2 + m (16B ktile stride for dual-fp8 ldweights).
    hq0 = nc.dram_tensor("hq0", [2, 128, QUART_COLS], f8, kind="ExternalInput")
    hh0 = nc.dram_tensor("hh0", [128, HALF_COLS], f8, kind="ExternalInput")
    hfull = nc.dram_tensor("hfull", [2, 128, FULL_COLS], f8, kind="ExternalInput")
    hh3 = nc.dram_tensor("hh3", [128, HALF_COLS], f8, kind="ExternalInput")
    hq3 = nc.dram_tensor("hq3", [2, 128, QUART_COLS], f8, kind="ExternalInput")
    out = nc.dram_tensor("out", [B_SHARD, 2 * H], f32, kind="ExternalOutput")

    def hs_dma(out_ap, in_ap):
        # All hs DMAs ride ONE HWDGE ring (sync) in consumption order: ring
        # FIFO makes blob completion order deterministic, and a single ring
        # still spreads each InstDMACopy across all 16 SDMA engines at full
        # ~430GB/s fabric rate. (Two rings with asymmetric loads let the
        # packet scheduler starve one ring and stall the PE.) Stores ride
        # the scalar ring so their DVE waits never stall an hs trigger.
        nc.sync.dma_start(out=out_ap, in_=in_ap)

    def blob_mms(t, hs_base, w_base, n_layers):
        """(lhsT, rhs_A, rhs_B) per (layer, dchunk) of one blob."""
        res = []
        wv = t[:, w_base : w_base + W_COLS].rearrange("p (i x) -> p i x", i=2)
        for l2 in range(n_layers):
            for d in range(N_DCH):
                lhsT = wv[:, :, d * 2 : d * 2 + 2]
                col0 = hs_base + (l2 * N_DCH + d) * 2 * H
                hsv = t[:, col0 : col0 + 2 * H].rearrange("p (i n) -> p i n", i=2)
                res.append((lhsT, hsv[:, :, 0:512], hsv[:, :, 512:H]))
        return res

    with tile.TileContext(nc) as tc:
        with (
            tc.tile_pool(name="hs_pool", bufs=2) as hs_pool,
            tc.tile_pool(name="hsf_pool", bufs=2) as hsf_pool,
            tc.tile_pool(name="hsq_pool", bufs=4) as hsq_pool,
            tc.tile_pool(name="warm_pool", bufs=1) as warm_pool,
            tc.tile_pool(name="out_pool", bufs=1) as out_pool,
            tc.tile_pool(name="psum", bufs=4, space="PSUM") as psum_pool,
        ):
            out_tile = out_pool.tile([2, B_SHARD * H], f32)

            # PE warmup: the PE runs its first ~8 matmuls at half rate
            # (clock ramp). Burn that ramp on dummy matmuls over garbage
            # SBUF before the first hs blob lands; their PSUM bank is
            # start=True-overwritten on reuse, so values never escape.
            warm = warm_pool.tile([128, 1056], f8, tag="warm")
            nc.vector.memset(warm[:], 0.0)
            warm_psum = psum_pool.tile([2, 512], f32, tag="psum_a")
            w_lhsT = warm[:, 1024:1056].rearrange("p (i x) -> p i x", i=2)[:, :, 0:2]
            w_rhs = warm[:, 0:1024].rearrange("p (i n) -> p i n", i=2)
            for _ in range(N_WARMUP):
                nc.tensor.matmul(warm_psum[:, :], w_lhsT, w_rhs,
                                 start=True, stop=True, perf_mode=DR)

            for b in range(B_SHARD):
                mm_args = []
                last_quarter = []
                if b == 0:
                    # Head: two single-layer quarters then a half, so the
                    # PE starts ~2us earlier than a full blob would allow.
                    for l in range(2):
                        t = hsq_pool.tile([128, QUART_COLS], f8, tag="hsq")
                        hs_dma(t[:], hq0[l])
                        mm_args += blob_mms(t, 0, QUART_HS, 1)
                    t = hs_pool.tile([128, HALF_COLS], f8, tag="hs")
                    hs_dma(t[:], hh0[:])
                    mm_args += blob_mms(t, 0, HALF_HS, 2)
                elif b < 3:
                    # Middle: one full-batch blob; its 12.3KB partition
                    # rows run the SDMA engines at max HBM efficiency.
                    t = hsf_pool.tile([128, FULL_COLS], f8, tag="hsf")
                    hs_dma(t[:], hfull[b - 1])
                    mm_args += blob_mms(t, 0, FULL_HS, 4)
                else:
                    # Tail: half then two quarters, so the last-arriving
                    # blob leaves only ~0.5us of matmuls.
                    t = hs_pool.tile([128, HALF_COLS], f8, tag="hs")
                    hs_dma(t[:], hh3[:])
                    mm_args += blob_mms(t, 0, HALF_HS, 2)
                    for l in range(2):
                        t = hsq_pool.tile([128, QUART_COLS], f8, tag="hsq")
                        hs_dma(t[:], hq3[l])
                        mms = blob_mms(t, 0, QUART_HS, 1)
                        if l == 1:
                            last_quarter = mms
                        else:
                            mm_args += mms
                    # Both output stores ride the sync HWDGE ring, emitted
                    # after every hs trigger so they never stall one; ring
                    # FIFO orders bulk before the drain-covered final store.
                    # Bulk (b0..b2) hides under b3's matmuls.
                    nc.scalar.dma_start(
                        out=out[0:3].rearrange("b (m h) -> m b h", m=2),
                        in_=out_tile[:, 0 : 3 * H].rearrange(
                            "m (b h) -> m b h", b=3
                        ),
                    )

                # Interleaved bank-A (N=512) / bank-B (N=256) groups in
                # separate PSUM banks. In the final quarter blob the B
                # group closes first so its copy overlaps A's last matmuls.
                psum_a = psum_pool.tile([2, 512], f32, tag="psum_a")
                psum_b = psum_pool.tile([2, H - 512], f32, tag="psum_b")
                na = nb = len(mm_args) + len(last_quarter)
                ia = ib = 0
                for lhsT, rhs_a, rhs_b in mm_args:
                    nc.tensor.matmul(psum_a[:, :], lhsT, rhs_a,
                                     start=ia == 0, stop=ia == na - 1,
                                     perf_mode=DR)
                    ia += 1
                    nc.tensor.matmul(psum_b[:, :], lhsT, rhs_b,
                                     start=ib == 0, stop=ib == nb - 1,
                                     perf_mode=DR)
                    ib += 1
                for lhsT, rhs_a, rhs_b in last_quarter:
                    nc.tensor.matmul(psum_b[:, :], lhsT, rhs_b,
                                     start=ib == 0, stop=ib == nb - 1,
                                     perf_mode=DR)
                    ib += 1
                for lhsT, rhs_a, rhs_b in last_quarter:
                    nc.tensor.matmul(psum_a[:, :], lhsT, rhs_a,
                                     start=ia == 0, stop=ia == na - 1,
                                     perf_mode=DR)
                    ia += 1
                nc.vector.tensor_copy(
                    out=out_tile[:, b * H + 512 : (b + 1) * H], in_=psum_b[:, :]
                )
                nc.vector.tensor_copy(
                    out=out_tile[:, b * H : b * H + 512], in_=psum_a[:, :]
                )

            # Final (b3) store: last DMACopy in program order; the exit
            # drain waits on its completion sem (see _fix_drain_waits).
            nc.scalar.dma_start(
                out=out[3:4].rearrange("b (m h) -> m b h", m=2),
                in_=out_tile[:, 3 * H : 4 * H].rearrange(
                    "m (b h) -> m b h", b=1
                ),
            )

    _fix_drain_waits(nc)
    return nc


def _fix_drain_waits(nc):
    """This container's walrus accepts only ONE sync wait per instruction.

    Two Tile-emitted multi-waits are redundant here and get stripped:

    1. Exit drains aggregate one wait per live semaphore; every semaphore
       except the final out-DMA's is transitively ordered before the drain
       (matmuls wait on hs DMAs -> PE; copies wait on PE -> DVE; the out
       DMAs wait on DVE and complete in sync-ring FIFO order ending with
       the final store), so drains keep only the final-store wait.
    2. The 12 HWDGE DMAs wrap the 8 DMAHW sem lanes, so the two out stores
       carry a lane-reuse guard wait next to their DVE wait. The guard is
       implied: the lane's previous hs DMA was already waited on by the
       matmuls that precede the DVE copies the store waits on.
    """
    import bass_rust

    f = nc.m.functions[0]
    # update-sem of the last DMACopy in program order (the final out store)
    last_dma_sem = None
    for bb in f.blocks:
        for ins in bb.instructions:
            if type(ins).__name__ == "InstDMACopy":
                ups = ins.sync_info.on_update
                if ups:
                    last_dma_sem = ups[-1].ant_name

    for bb in f.blocks:
        for ins in bb.instructions:
            nm = type(ins).__name__
            si = ins.sync_info
            if si is None:
                continue
            waits = list(si.on_wait)
            if len(waits) <= 1:
                continue
            if nm == "InstDrain":
                keep = [w for w in waits if w.ant_name == last_dma_sem]
            elif nm == "InstDMACopy":
                keep = [w for w in waits if not w.ant_name.startswith("DMAHW")]
            else:
                continue
            assert len(keep) == 1, (nm, [w.ant_name for w in waits])
            ins.sync_info = bass_rust.SyncInfo(
                on_wait=keep, on_update=list(si.on_update)
            )


def _host_masks(input_ids, attention_mask, token_type_ids):
    ids = np.asarray(input_ids)
    am = np.asarray(attention_mask)
    tt = np.asarray(token_type_ids)

    not_pad = ids != PAD_ID
    before_pad = np.cumprod(not_pad.astype(np.int64), axis=1).astype(bool)
    valid = before_pad & (ids != CLS_ID) & (ids != SEP_ID) & (am == 1)
    term = valid & (tt == 0)
    text = valid & (tt == 1)
    masks = np.stack([term, text], axis=-1)  # [B, S, 2] bool
    counts = masks.sum(axis=1).astype(np.float64)  # [B, 2]
    return masks, counts


def _diffused_fp8(hs4, masks):
    """Quantize to fp8_e4m3 with error diffusion along each group's (l,s)
    reduction chain: the rounding residual of each masked element is carried
    into the next masked element of the same (b, h, group) chain, so each
    group's quantization errors telescope to ~1 ulp instead of a sqrt(N)
    random walk. DIFF_CHAINS stripes s into parallel chains (vectorizing the
    host loop) at a sqrt(DIFF_CHAINS) error cost; measured group-sum rel err
    ~2.4e-3 vs the 2e-2 gate. Device-side sum order doesn't matter -- only
    the group SUM of the quantized values.
    """
    import ml_dtypes

    F8 = ml_dtypes.float8_e4m3
    K = DIFF_CHAINS
    q = np.empty(hs4.shape, dtype=F8)  # [4, B, S, H]
    gt_all = masks[:, :, 0]  # [B, S]
    gx_all = masks[:, :, 1]
    carry_t = np.zeros((K, B, H), dtype=np.float32)
    carry_x = np.zeros((K, B, H), dtype=np.float32)
    for l in range(N_LAYERS):
        for j in range(S // K):
            sblk = slice(j * K, (j + 1) * K)
            gt = gt_all[:, sblk].T[:, :, None]  # [K, B, 1]
            gx = gx_all[:, sblk].T[:, :, None]
            t = hs4[l, :, sblk, :].transpose(1, 0, 2) + np.where(
                gt, carry_t, carry_x
            )  # [K, B, H]
            qv = t.astype(F8)
            q[l, :, sblk, :] = qv.transpose(1, 0, 2)
            resid = t - qv.astype(np.float32)
            carry_t = np.where(gt, resid, carry_t)
            carry_x = np.where(gx, resid, carry_x)
    return q


def kernel(hidden_states, input_ids, attention_mask, token_type_ids):
    from concourse.bass_utils import run_bass_kernel_spmd

    hs_full = np.asarray(hidden_states)
    masks, counts = _host_masks(input_ids, attention_mask, token_type_ids)

    q = _diffused_fp8(hs_full[L - N_LAYERS :].astype(np.float32), masks)
    F8 = q.dtype

    # Half-blobs [B, hf, p, ((l2 d) i n)] and quarter-blobs [B, l, p, (d i n)]
    # with s = d*256 + i*128 + p.
    half = np.empty((B, 2, 128, HALF_COLS), dtype=F8)
    half[:, :, :, :HALF_HS] = (
        q.reshape(2, 2, B, N_DCH, 2, 128, H)
        .transpose(2, 0, 5, 1, 3, 4, 6)
        .reshape(B, 2, 128, HALF_HS)
    )
    quart = np.empty((B, N_LAYERS, 128, QUART_COLS), dtype=F8)
    quart[:, :, :, :QUART_HS] = (
        q.reshape(N_LAYERS, B, N_DCH, 2, 128, H)
        .transpose(1, 0, 4, 2, 3, 5)
        .reshape(B, N_LAYERS, 128, QUART_HS)
    )
    full = np.empty((B, 128, FULL_COLS), dtype=F8)
    full[:, :, :FULL_HS] = (
        q.reshape(N_LAYERS, B, N_DCH, 2, 128, H)
        .transpose(1, 4, 0, 2, 3, 5)
        .reshape(B, 128, FULL_HS)
    )
    wv = np.zeros((B, 128, 2, 16), dtype=F8)
    wv[:, :, :, 0:4] = (
        masks.reshape(B, N_DCH, 2, 128, 2)
        .transpose(0, 3, 2, 1, 4)          # (b, p, i, d, m)
        .reshape(B, 128, 2, 4)
        .astype(F8)
    )
    wv = wv.reshape(B, 128, W_COLS)
    half[:, :, :, HALF_HS:] = wv[:, None, :, :]
    quart[:, :, :, QUART_HS:] = wv[:, None, :, :]
    full[:, :, FULL_HS:] = wv

    in_maps = [
        {
            "hq0": quart[i * B_SHARD, 0:2],
            "hh0": half[i * B_SHARD, 1],
            "hfull": full[i * B_SHARD + 1 : i * B_SHARD + 3],
            "hh3": half[i * B_SHARD + 3, 0],
            "hq3": quart[i * B_SHARD + 3, 2:4],
        }
        for i in range(N_CORES)
    ]

    if "nc" not in _CACHED:
        _CACHED["nc"] = _build_bass()
    nc = _CACHED["nc"]

    trace = os.environ.get("KERNEL_TRACE", "0") == "1"
    if trace:
        _install_ntff_hook_shim()
    tmpdir = os.environ.get("KERNEL_TMPDIR") or None
    res = run_bass_kernel_spmd(
        nc, in_maps, core_ids=list(range(N_CORES)), trace=trace, tmpdir=tmpdir
    )
    kernel.last_results = res

    acc = np.concatenate([r["out"] for r in res.results], axis=0)  # [B, 2H]
    # Apply the masked-mean normalization (exact f64 scale, mirrors the
    # reference's sum/count including inf/nan semantics for count==0).
    with np.errstate(divide="ignore", invalid="ignore"):
        scale = 1.0 / (N_LAYERS * counts)  # [B, 2]
    out = acc.reshape(B, 2, H) * scale[:, :, None]
    return out.reshape(B, 2 * H).astype(np.float32)


def _install_ntff_hook_shim():
    """The container's antenv stub lacks axon_hooks, which silently disables
    NTFF profiling under trace=True. Recreate it: a tiny get/set registry plus
    the ctypes hook into libaxon_pjrt.so (same as trn_boot's installer)."""
    import contextlib
    import ctypes
    import sys
    import types

    if "antenv.axon_hooks" in sys.modules:
        return
    so_path = "/opt/axon/libaxon_pjrt.so"
    try:
        lib = ctypes.CDLL(so_path)
    except OSError:
        return
    if not hasattr(lib, "axon_start_nrt_profile"):
        return
    lib.axon_start_nrt_profile.argtypes = [
        ctypes.POINTER(ctypes.c_int64),
        ctypes.c_size_t,
    ]
    lib.axon_start_nrt_profile.restype = ctypes.c_int64
    lib.axon_stop_nrt_profile.argtypes = [ctypes.c_char_p]
    lib.axon_stop_nrt_profile.restype = ctypes.c_int64

    @contextlib.contextmanager
    def _hook(output_dir, device_ids):
        import jax

        jax.devices()
        if device_ids:
            ids = (ctypes.c_int64 * len(device_ids))(*device_ids)
            rc = lib.axon_start_nrt_profile(ids, len(device_ids))
        else:
            rc = lib.axon_start_nrt_profile(None, 0)
        if rc != 0:
            raise RuntimeError(f"axon_start_nrt_profile rc={rc}")
        try:
            yield
        finally:
            n = lib.axon_stop_nrt_profile(str(output_dir).encode())
            print(f"profile: {n} file(s) written to {output_dir}", file=sys.stderr)

    mod = types.ModuleType("antenv.axon_hooks")
    _state = {"hook": _hook}
    mod.set_axon_ntff_profile_hook = lambda h: _state.__setitem__("hook", h)
    mod.get_axon_ntff_profile_hook = lambda: _state["hook"]
    sys.modules["antenv.axon_hooks"] = mod
    import antenv

    antenv.axon_hooks = mod


# revision 10
# speedup vs baseline: 1.1922x; 1.0878x over previous
"""Bass/Trainium2 kernel for nn_CustomPooling (segment_reduce, masked mean pooling).

Reference computation:
  hs = mean(hidden_states[-4:], axis=0)                      # [B,S,H]
  valid = before_pad & ~CLS & ~SEP & attention
  term_mean = sum_s(hs * term_mask) / sum(term_mask)         # [B,H]
  text_mean = sum_s(hs * text_mask) / sum(text_mask)         # [B,H]
  out = concat([term_mean, text_mean], -1)                   # [B,2H]

Strategy:
  - Only the last 4 layers are ever read (201MB of the 654MB input).
  - The [B,S] int masks reduce to binary {0,1} per-(b,s) weights; the
    1/(4*count) scale is applied to the tiny [B,2H] result on the host, so
    the device work is a pure masked sum over (layer, s):
      acc[b, m*H + h] = sum_{l,s} hs[l,b,s,h] * mask[b,s,m]
  - That reduction is a TensorE matmul with the [128,2,2] binary mask slice
    stationary and hs [128,2,N] moving in fp8 DoubleRow perf mode (256-deep
    contraction per matmul, 2 elem/partition/cycle), accumulated in fp32
    PSUM over 2 double-s-chunks x 4 layers. Data ships as fp8_e4m3 ({0,1}
    masks exact); quantization uses error diffusion along each group's
    (l,s) reduction chain so group sums keep ~3e-3 relative accuracy. This
    halves DMA bytes vs fp16 (the kernel is HBM/fabric-DMA-bound at
    ~430GB/s/core) and doubles PE throughput.
  - Data parallel over B: 8 cores x 4 batches, no collectives.
  - Host pre-swizzles each (batch, layer-pair) into one contiguous
    [128, 6152] fp8 blob (its own weight copy appended) so each tile is
    ONE DMA and every matmul waits on exactly one DMA semaphore (this
    toolchain accepts a single sync wait per instruction). All hs DMAs
    ride the sync HWDGE ring in consumption order (one ring already runs
    all 16 SDMA engines at the ~430GB/s fabric rate, and ring FIFO makes
    arrival order deterministic); both output stores ride the scalar ring
    (HWDGE, ~0.6us completion receipt vs ~2us SWDGE), with ring FIFO
    ordering the bulk store before the final store the exit drain waits
    on. Dummy warmup matmuls absorb the PE's ~4us half-rate clock ramp
    before the first blob lands.
"""

import os

import numpy as np

# Hardcoded problem shape (kernel.py must be self-contained).
L, B, S, H = 13, 32, 512, 768
N_LAYERS = 4          # layers -4..-1
N_CORES = 8
B_SHARD = B // N_CORES          # 4 batches per core
N_DCH = S // 256                # 2 double-s-chunks of 256 (DoubleRow contraction)
# Mask region: col = i*16 + d*2 + m (32B; dual-fp8 ldweights needs the
# ktile stride of the stationary AP 16B-aligned), zeros elsewhere.
W_COLS = 32
# Bulk batches (0..2) ship as two half-blobs (2 layers each); the tail
# batch ships as four quarter-blobs (1 layer) so the last-arriving tile
# only needs ~0.7us of matmuls after the final DMA lands.
HALF_HS = 2 * N_DCH * 2 * H              # 6144
HALF_COLS = HALF_HS + W_COLS             # 6152
QUART_HS = N_DCH * 2 * H                 # 3072
QUART_COLS = QUART_HS + W_COLS           # 3104
FULL_HS = 4 * N_DCH * 2 * H              # 12288
FULL_COLS = FULL_HS + W_COLS             # 12320 (12.3KB rows: max DMA rate)
CLS_ID, SEP_ID, PAD_ID = 101, 102, 0
DIFF_CHAINS = 8       # parallel error-diffusion chains per (b,h,group)
N_WARMUP = 18         # PE warmup: bridge clock ramp AND delay real start

_CACHED = {}


def _build_bass():
    import concourse.bass as bass
    import concourse.tile as tile
    from concourse import mybir

    f8 = mybir.dt.float8e4
    f32 = mybir.dt.float32
    DR = mybir.MatmulPerfMode.DoubleRow
    nc = bass.Bass()

    # Per-core inputs (host-preswizzled fp8 blobs, masks appended to each):
    #   hsa[b, hf, p, (l2 d) * 1536 + i*768 + n], b in 0..2  (two half-blobs)
    #   hsb[l, p, d*1536 + i*768 + n]                        (batch 3, per layer)
    # where row p and ktile i encode s = d*256 + i*128 + p. Mask cols at
    # hs_end + i*16 + d*2 + m (16B ktile stride for dual-fp8 ldweights).
    hfull = nc.dram_tensor("hfull", [2, 128, FULL_COLS], f8, kind="ExternalInput")
    hhalf = nc.dram_tensor("hhalf", [3, 128, HALF_COLS], f8, kind="ExternalInput")
    hq3 = nc.dram_tensor("hq3", [2, 128, QUART_COLS], f8, kind="ExternalInput")
    out = nc.dram_tensor("out", [B_SHARD, 2 * H], f32, kind="ExternalOutput")

    def hs_dma(out_ap, in_ap):
        # All hs DMAs ride ONE HWDGE ring (sync) in consumption order: ring
        # FIFO makes blob completion order deterministic, and a single ring
        # still spreads each InstDMACopy across all 16 SDMA engines at full
        # ~430GB/s fabric rate. (Two rings with asymmetric loads let the
        # packet scheduler starve one ring and stall the PE.) Stores ride
        # the scalar ring so their DVE waits never stall an hs trigger.
        nc.sync.dma_start(out=out_ap, in_=in_ap)

    def blob_mms(t, hs_base, w_base, n_layers):
        """(lhsT, rhs_A, rhs_B) per (layer, dchunk) of one blob."""
        res = []
        wv = t[:, w_base : w_base + W_COLS].rearrange("p (i x) -> p i x", i=2)
        for l2 in range(n_layers):
            for d in range(N_DCH):
                lhsT = wv[:, :, d * 2 : d * 2 + 2]
                col0 = hs_base + (l2 * N_DCH + d) * 2 * H
                hsv = t[:, col0 : col0 + 2 * H].rearrange("p (i n) -> p i n", i=2)
                res.append((lhsT, hsv[:, :, 0:512], hsv[:, :, 512:H]))
        return res

    with tile.TileContext(nc) as tc:
        with (
            tc.tile_pool(name="hs_pool", bufs=3) as hs_pool,
            tc.tile_pool(name="hsf_pool", bufs=2) as hsf_pool,
            tc.tile_pool(name="hsq_pool", bufs=2) as hsq_pool,
            tc.tile_pool(name="warm_pool", bufs=1) as warm_pool,
            tc.tile_pool(name="out_pool", bufs=1) as out_pool,
            tc.tile_pool(name="psum", bufs=4, space="PSUM") as psum_pool,
        ):
            out_tile = out_pool.tile([2, B_SHARD * H], f32)

            # PE warmup: the PE runs its first ~8 matmuls at half rate
            # (clock ramp). Burn that ramp on dummy matmuls over garbage
            # SBUF before the first hs blob lands; their PSUM bank is
            # start=True-overwritten on reuse, so values never escape.
            warm = warm_pool.tile([128, 1056], f8, tag="warm")
            nc.vector.memset(warm[:], 0.0)
            warm_psum = psum_pool.tile([2, 512], f32, tag="psum_a")
            w_lhsT = warm[:, 1024:1056].rearrange("p (i x) -> p i x", i=2)[:, :, 0:2]
            w_rhs = warm[:, 0:1024].rearrange("p (i n) -> p i n", i=2)
            for _ in range(N_WARMUP):
                nc.tensor.matmul(warm_psum[:, :], w_lhsT, w_rhs,
                                 start=True, stop=True, perf_mode=DR)

            for b in range(B_SHARD):
                mm_args = []
                last_quarter = []
                if b < 2:
                    # Head/middle: full-batch blobs; their 12.3KB partition
                    # rows run the SDMA engines at max HBM efficiency. The
                    # PE intentionally starts late (long warmup) so it runs
                    # one contiguous full-clock burst ending at stream end.
                    t = hsf_pool.tile([128, FULL_COLS], f8, tag="hsf")
                    hs_dma(t[:], hfull[b])
                    mm_args += blob_mms(t, 0, FULL_HS, 4)
                elif b == 2:
                    for hf in range(2):
                        t = hs_pool.tile([128, HALF_COLS], f8, tag="hs")
                        hs_dma(t[:], hhalf[hf])
                        mm_args += blob_mms(t, 0, HALF_HS, 2)
                else:
                    # Tail: half then two quarters, so the last-arriving
                    # blob leaves only ~0.5us of matmuls.
                    t = hs_pool.tile([128, HALF_COLS], f8, tag="hs")
                    hs_dma(t[:], hhalf[2])
                    mm_args += blob_mms(t, 0, HALF_HS, 2)
                    for l in range(2):
                        t = hsq_pool.tile([128, QUART_COLS], f8, tag="hsq")
                        hs_dma(t[:], hq3[l])
                        mms = blob_mms(t, 0, QUART_HS, 1)
                        if l == 1:
                            last_quarter = mms
                        else:
                            mm_args += mms
                    # Both output stores ride the sync HWDGE ring, emitted
                    # after every hs trigger so they never stall one; ring
                    # FIFO orders bulk before the drain-covered final store.
                    # Bulk (b0..b2) hides under b3's matmuls.
                    nc.scalar.dma_start(
                        out=out[0:3].rearrange("b (m h) -> m b h", m=2),
                        in_=out_tile[:, 0 : 3 * H].rearrange(
                            "m (b h) -> m b h", b=3
                        ),
                    )

                # Interleaved bank-A (N=512) / bank-B (N=256) groups in
                # separate PSUM banks. In the final quarter blob the B
                # group closes first so its copy overlaps A's last matmuls.
                psum_a = psum_pool.tile([2, 512], f32, tag="psum_a")
                psum_b = psum_pool.tile([2, H - 512], f32, tag="psum_b")
                na = nb = len(mm_args) + len(last_quarter)
                ia = ib = 0
                for lhsT, rhs_a, rhs_b in mm_args:
                    nc.tensor.matmul(psum_a[:, :], lhsT, rhs_a,
                                     start=ia == 0, stop=ia == na - 1,
                                     perf_mode=DR)
                    ia += 1
                    nc.tensor.matmul(psum_b[:, :], lhsT, rhs_b,
                                     start=ib == 0, stop=ib == nb - 1,
                                     perf_mode=DR)
                    ib += 1
                for lhsT, rhs_a, rhs_b in last_quarter:
                    nc.tensor.matmul(psum_a[:, :], lhsT, rhs_a,
                                     start=ia == 0, stop=ia == na - 1,
                                     perf_mode=DR)
                    ia += 1
                for lhsT, rhs_a, rhs_b in last_quarter:
                    nc.tensor.matmul(psum_b[:, :], lhsT, rhs_b,
                                     start=ib == 0, stop=ib == nb - 1,
                                     perf_mode=DR)
                    ib += 1
                nc.vector.tensor_copy(
                    out=out_tile[:, b * H : b * H + 512], in_=psum_a[:, :]
                )
                nc.vector.tensor_copy(
                    out=out_tile[:, b * H + 512 : (b + 1) * H], in_=psum_b[:, :]
                )

            # Final (b3) store: last DMACopy in program order; the exit
            # drain waits on its completion sem (see _fix_drain_waits).
            nc.scalar.dma_start(
                out=out[3:4].rearrange("b (m h) -> m b h", m=2),
                in_=out_tile[:, 3 * H : 4 * H].rearrange(
                    "m (b h) -> m b h", b=1
                ),
            )

    _fix_drain_waits(nc)
    return nc


def _fix_drain_waits(nc):
    """This container's walrus accepts only ONE sync wait per instruction.

    Two Tile-emitted multi-waits are redundant here and get stripped:

    1. Exit drains aggregate one wait per live semaphore; every semaphore
       except the final out-DMA's is transitively ordered before the drain
       (matmuls wait on hs DMAs -> PE; copies wait on PE -> DVE; the out
       DMAs wait on DVE and complete in sync-ring FIFO order ending with
       the final store), so drains keep only the final-store wait.
    2. The 12 HWDGE DMAs wrap the 8 DMAHW sem lanes, so the two out stores
       carry a lane-reuse guard wait next to their DVE wait. The guard is
       implied: the lane's previous hs DMA was already waited on by the
       matmuls that precede the DVE copies the store waits on.
    """
    import bass_rust

    f = nc.m.functions[0]
    # update-sem of the last DMACopy in program order (the final out store)
    last_dma_sem = None
    for bb in f.blocks:
        for ins in bb.instructions:
            if type(ins).__name__ == "InstDMACopy":
                ups = ins.sync_info.on_update
                if ups:
                    last_dma_sem = ups[-1].ant_name

    for bb in f.blocks:
        for ins in bb.instructions:
            nm = type(ins).__name__
            si = ins.sync_info
            if si is None:
                continue
            waits = list(si.on_wait)
            if len(waits) <= 1:
                continue
            if nm == "InstDrain":
                keep = [w for w in waits if w.ant_name == last_dma_sem]
            elif nm == "InstDMACopy":
                keep = [w for w in waits if not w.ant_name.startswith("DMAHW")]
            else:
                continue
            assert len(keep) == 1, (nm, [w.ant_name for w in waits])
            ins.sync_info = bass_rust.SyncInfo(
                on_wait=keep, on_update=list(si.on_update)
            )


def _host_masks(input_ids, attention_mask, token_type_ids):
    ids = np.asarray(input_ids)
    am = np.asarray(attention_mask)
    tt = np.asarray(token_type_ids)

    not_pad = ids != PAD_ID
    before_pad = np.cumprod(not_pad.astype(np.int64), axis=1).astype(bool)
    valid = before_pad & (ids != CLS_ID) & (ids != SEP_ID) & (am == 1)
    term = valid & (tt == 0)
    text = valid & (tt == 1)
    masks = np.stack([term, text], axis=-1)  # [B, S, 2] bool
    counts = masks.sum(axis=1).astype(np.float64)  # [B, 2]
    return masks, counts


def _diffused_fp8(hs4, masks):
    """Quantize to fp8_e4m3 with error diffusion along each group's (l,s)
    reduction chain: the rounding residual of each masked element is carried
    into the next masked element of the same (b, h, group) chain, so each
    group's quantization errors telescope to ~1 ulp instead of a sqrt(N)
    random walk. DIFF_CHAINS stripes s into parallel chains (vectorizing the
    host loop) at a sqrt(DIFF_CHAINS) error cost; measured group-sum rel err
    ~2.4e-3 vs the 2e-2 gate. Device-side sum order doesn't matter -- only
    the group SUM of the quantized values.
    """
    import ml_dtypes

    F8 = ml_dtypes.float8_e4m3
    K = DIFF_CHAINS
    q = np.empty(hs4.shape, dtype=F8)  # [4, B, S, H]
    gt_all = masks[:, :, 0]  # [B, S]
    gx_all = masks[:, :, 1]
    carry_t = np.zeros((K, B, H), dtype=np.float32)
    carry_x = np.zeros((K, B, H), dtype=np.float32)
    for l in range(N_LAYERS):
        for j in range(S // K):
            sblk = slice(j * K, (j + 1) * K)
            gt = gt_all[:, sblk].T[:, :, None]  # [K, B, 1]
            gx = gx_all[:, sblk].T[:, :, None]
            t = hs4[l, :, sblk, :].transpose(1, 0, 2) + np.where(
                gt, carry_t, carry_x
            )  # [K, B, H]
            qv = t.astype(F8)
            q[l, :, sblk, :] = qv.transpose(1, 0, 2)
            resid = t - qv.astype(np.float32)
            carry_t = np.where(gt, resid, carry_t)
            carry_x = np.where(gx, resid, carry_x)
    return q


def kernel(hidden_states, input_ids, attention_mask, token_type_ids):
    from concourse.bass_utils import run_bass_kernel_spmd

    hs_full = np.asarray(hidden_states)
    masks, counts = _host_masks(input_ids, attention_mask, token_type_ids)

    q = _diffused_fp8(hs_full[L - N_LAYERS :].astype(np.float32), masks)
    F8 = q.dtype

    # Half-blobs [B, hf, p, ((l2 d) i n)] and quarter-blobs [B, l, p, (d i n)]
    # with s = d*256 + i*128 + p.
    half = np.empty((B, 2, 128, HALF_COLS), dtype=F8)
    half[:, :, :, :HALF_HS] = (
        q.reshape(2, 2, B, N_DCH, 2, 128, H)
        .transpose(2, 0, 5, 1, 3, 4, 6)
        .reshape(B, 2, 128, HALF_HS)
    )
    quart = np.empty((B, N_LAYERS, 128, QUART_COLS), dtype=F8)
    quart[:, :, :, :QUART_HS] = (
        q.reshape(N_LAYERS, B, N_DCH, 2, 128, H)
        .transpose(1, 0, 4, 2, 3, 5)
        .reshape(B, N_LAYERS, 128, QUART_HS)
    )
    full = np.empty((B, 128, FULL_COLS), dtype=F8)
    full[:, :, :FULL_HS] = (
        q.reshape(N_LAYERS, B, N_DCH, 2, 128, H)
        .transpose(1, 4, 0, 2, 3, 5)
        .reshape(B, 128, FULL_HS)
    )
    wv = np.zeros((B, 128, 2, 16), dtype=F8)
    wv[:, :, :, 0:4] = (
        masks.reshape(B, N_DCH, 2, 128, 2)
        .transpose(0, 3, 2, 1, 4)          # (b, p, i, d, m)
        .reshape(B, 128, 2, 4)
        .astype(F8)
    )
    wv = wv.reshape(B, 128, W_COLS)
    half[:, :, :, HALF_HS:] = wv[:, None, :, :]
    quart[:, :, :, QUART_HS:] = wv[:, None, :, :]
    full[:, :, FULL_HS:] = wv

    in_maps = [
        {
            "hfull": full[i * B_SHARD : i * B_SHARD + 2],
            "hhalf": np.stack(
                [half[i * B_SHARD + 2, 0], half[i * B_SHARD + 2, 1],
                 half[i * B_SHARD + 3, 0]]
            ),
            "hq3": quart[i * B_SHARD + 3, 2:4],
        }
        for i in range(N_CORES)
    ]

    if "nc" not in _CACHED:
        _CACHED["nc"] = _build_bass()
    nc = _CACHED["nc"]

    trace = os.environ.get("KERNEL_TRACE", "0") == "1"
    if trace:
        _install_ntff_hook_shim()
    tmpdir = os.environ.get("KERNEL_TMPDIR") or None
    res = run_bass_kernel_spmd(
        nc, in_maps, core_ids=list(range(N_CORES)), trace=trace, tmpdir=tmpdir
    )
    kernel.last_results = res

    acc = np.concatenate([r["out"] for r in res.results], axis=0)  # [B, 2H]
    # Apply the masked-mean normalization (exact f64 scale, mirrors the
    # reference's sum/count including inf/nan semantics for count==0).
    with np.errstate(divide="ignore", invalid="ignore"):
        scale = 1.0 / (N_LAYERS * counts)  # [B, 2]
    out = acc.reshape(B, 2, H) * scale[:, :, None]
    return out.reshape(B, 2 * H).astype(np.float32)


def _install_ntff_hook_shim():
    """The container's antenv stub lacks axon_hooks, which silently disables
    NTFF profiling under trace=True. Recreate it: a tiny get/set registry plus
    the ctypes hook into libaxon_pjrt.so (same as trn_boot's installer)."""
    import contextlib
    import ctypes
    import sys
    import types

    if "antenv.axon_hooks" in sys.modules:
        return
    so_path = "/opt/axon/libaxon_pjrt.so"
    try:
        lib = ctypes.CDLL(so_path)
    except OSError:
        return
    if not hasattr(lib, "axon_start_nrt_profile"):
        return
    lib.axon_start_nrt_profile.argtypes = [
        ctypes.POINTER(ctypes.c_int64),
        ctypes.c_size_t,
    ]
    lib.axon_start_nrt_profile.restype = ctypes.c_int64
    lib.axon_stop_nrt_profile.argtypes = [ctypes.c_char_p]
    lib.axon_stop_nrt_profile.restype = ctypes.c_int64

    @contextlib.contextmanager
    def _hook(output_dir, device_ids):
        import jax

        jax.devices()
        if device_ids:
            ids = (ctypes.c_int64 * len(device_ids))(*device_ids)
            rc = lib.axon_start_nrt_profile(ids, len(device_ids))
        else:
            rc = lib.axon_start_nrt_profile(None, 0)
        if rc != 0:
            raise RuntimeError(f"axon_start_nrt_profile rc={rc}")
        try:
            yield
        finally:
            n = lib.axon_stop_nrt_profile(str(output_dir).encode())
            print(f"profile: {n} file(s) written to {output_dir}", file=sys.stderr)

    mod = types.ModuleType("antenv.axon_hooks")
    _state = {"hook": _hook}
    mod.set_axon_ntff_profile_hook = lambda h: _state.__setitem__("hook", h)
    mod.get_axon_ntff_profile_hook = lambda: _state["hook"]
    sys.modules["antenv.axon_hooks"] = mod
    import antenv

    antenv.axon_hooks = mod
